# revision 1
# baseline (speedup 1.0000x reference)
"""Trainium2 Bass kernel for nn_CapsuleLayer_9852654977072.

The reference module collapses mathematically: the routing loop's coupling
logits `b` stay zero (faithfully-reproduced bug in the original torch code),
so routing coefficients are a fixed spatial map r(h,w) = 1/(8*cnt(h,w)) where
cnt is the 5x5 box-count inside the image. The whole module is therefore:

    p = conv2d(u as [N,64,H,W], Wd as [128,64,5,5], pad=2) * s(h,w)
    v = squash_z1(p)   # groups of 16 channels
    out[n,t1,z1,h,w] = v

Device strategy (8 cores, SPMD): shard (batch n in 0..3) x (row-half in 0..1).
Each core computes all 128 output channels for 64 rows of one image.

Conv: inputs shipped as XA/XC [128, 68, 132] whose partition halves hold u
shifted by (+0row,+1row) and (+2row+0col,+2row+1col) respectively, columns
padded by 2. Per 4-row block, 13 PSUM-accumulated fp32r matmuls (N=512, full
PE rate) cover all 25 taps: 10 XA row-pairs + 2 XC col-pairs + 1 K=64 single.

Squash: square (ACT) -> block-diag matmul (n2 over z1) -> factor on
8-partition tiles with the spatial scale folded in via a s^2 map
(F = y/((1+y)sqrt(y_raw+eps)), y = s^2*y_raw) -> expand matmul -> v = p*F.
"""

import numpy as np

T0, Z0, T1, Z1, KK, PAD = 4, 16, 8, 16, 5, 2
N, H, W_SP = 4, 128, 128
CIN, COUT = T0 * Z0, T1 * Z1  # 64, 128
N_CORES = 8
ROWS = 64          # output rows per core
XROWS = 68         # input rows incl. halo
XCOLS = 132        # 128 + 2*PAD
BLK = 4            # output rows per block
N_BLKS = ROWS // BLK

# conv matmul j -> (source, row_off, col_off); weights match in _weight_tiles
_MM_SLICES = (
    [('XA', dy + 2, dx + 2) for dy in (-2, 0) for dx in (-2, -1, 0, 1, 2)]
    + [('XC', 2, 0), ('XC', 2, 2), ('XC', 2, 4)]
)

_CACHE = {}


def _weight_tiles(W):
    Wd = W.transpose(1, 0, 2, 3, 4).reshape(COUT, CIN, KK, KK)
    wl = np.zeros((128, 13, 128), np.float32)  # [k, j, m]
    j = 0
    for dy in (-2, 0):
        for dx in (-2, -1, 0, 1, 2):
            wl[0:64, j, :] = Wd[:, :, dy + 2, dx + 2].T
            wl[64:128, j, :] = Wd[:, :, dy + 3, dx + 2].T
            j += 1
    for dx0 in (-2, 0):
        wl[0:64, j, :] = Wd[:, :, 4, dx0 + 2].T
        wl[64:128, j, :] = Wd[:, :, 4, dx0 + 3].T
        j += 1
    wl[0:64, j, :] = Wd[:, :, 4, 4].T  # single tap (2,2) on lo partitions
    return wl


def _inputs_core(x, half):
    """x: [64, H, W] one image channel-major. Returns XA, XC [128, 68, 132]."""
    base = half * 64 - 2
    XA = np.zeros((128, XROWS, XCOLS), np.float32)
    XC = np.zeros((128, XROWS, XCOLS), np.float32)

    def fill(dst, roff, c0, c1):
        lo, hi = max(0, -(base + roff)), min(XROWS, H - base - roff)
        dst[:, lo:hi, c0:c1] = x[:, base + roff + lo:base + roff + hi, :]

    fill(XA[0:64], 0, 2, 130)
    fill(XA[64:128], 1, 2, 130)
    fill(XC[0:64], 2, 2, 130)
    fill(XC[64:128], 2, 1, 129)
    return XA, XC


def _s2_map(half):
    idx = np.arange(H)
    cnt = (np.minimum(idx + 2, H - 1) - np.maximum(idx - 2, 0) + 1).astype(np.float64)
    s = 1.0 / (8.0 * cnt[:, None] * cnt[None, :])  # [H, W]
    s = s[half * 64:(half + 1) * 64, :]
    return np.ascontiguousarray((s * s).astype(np.float32).reshape(1, ROWS * 128))


def _block_diag():
    bd = np.zeros((128, 8), np.float32)
    bd[np.arange(128), np.arange(128) // 16] = 1.0
    return bd


def build_nc(reps=1):
    import concourse.bass as bass
    import concourse.bacc as bacc
    import concourse.mybir as mybir
    import concourse.tile as tile

    f32 = mybir.dt.float32
    f32r = mybir.dt.float32r
    AF = mybir.ActivationFunctionType

    nc = bacc.Bacc(None, target_bir_lowering=False)
    xa_d = nc.dram_tensor("xa", [128, XROWS * XCOLS], f32r, kind="ExternalInput")
    xc_d = nc.dram_tensor("xc", [128, XROWS * XCOLS], f32r, kind="ExternalInput")
    wl_d = nc.dram_tensor("wl", [128, 13 * 128], f32r, kind="ExternalInput")
    bd_d = nc.dram_tensor("bd", [128, 8], f32r, kind="ExternalInput")
    ex_d = nc.dram_tensor("ex", [8, 128], f32r, kind="ExternalInput")
    s2_d = nc.dram_tensor("s2", [1, ROWS * 128], f32, kind="ExternalInput")
    out_d = nc.dram_tensor("out", [128, ROWS * 128], f32, kind="ExternalOutput")

    with tile.TileContext(nc) as tc:
        with (
            tc.tile_pool(name="consts", bufs=1) as consts,
            tc.tile_pool(name="work", bufs=4) as work,
            tc.tile_pool(name="small", bufs=6) as small,
            tc.tile_pool(name="pp", bufs=3, space="PSUM") as pp,
            tc.tile_pool(name="pf", bufs=2, space="PSUM") as pf,
            tc.tile_pool(name="py", bufs=2, space="PSUM") as py,
        ):
            wl = consts.tile([128, 13, 128], f32r)
            nc.sync.dma_start(
                out=wl, in_=wl_d.ap().rearrange("p (j m) -> p j m", m=128))
            bd = consts.tile([128, 8], f32r)
            nc.sync.dma_start(out=bd, in_=bd_d.ap())
            ex = consts.tile([8, 128], f32r)
            nc.sync.dma_start(out=ex, in_=ex_d.ap())
            s2_sb = consts.tile([8, ROWS, 128], f32)
            s2_ap = s2_d.ap()
            nc.sync.dma_start(
                out=s2_sb,
                in_=bass.AP(tensor=s2_ap.tensor, offset=s2_ap.offset,
                            ap=[[0, 8], [128, ROWS], [1, 128]]))
            eps_t = consts.tile([8, 1], f32)
            nc.vector.memset(eps_t[:], 1e-9)

            xa = consts.tile([128, XROWS, XCOLS], f32r)
            xc = consts.tile([128, XROWS, XCOLS], f32r)
            xa_src = xa_d.ap().rearrange("p (r c) -> p r c", c=XCOLS)
            xc_src = xc_d.ap().rearrange("p (r c) -> p r c", c=XCOLS)
            for c0 in range(0, XROWS, 17):
                nc.sync.dma_start(
                    out=xa[:, c0:c0 + 17, :], in_=xa_src[:, c0:c0 + 17, :])
                nc.sync.dma_start(
                    out=xc[:, c0:c0 + 17, :], in_=xc_src[:, c0:c0 + 17, :])

            out_v = out_d.ap().rearrange("p (r c) -> p r c", c=128)

            import contextlib
            loop_ctx = (tc.For_i(0, reps, 1,
                                 hint_engines=(mybir.EngineType.PE,
                                               mybir.EngineType.DVE,
                                               mybir.EngineType.Activation,
                                               mybir.EngineType.Pool,
                                               mybir.EngineType.SP))
                        if reps > 1 else contextlib.nullcontext())
            def stage0(blk):
                r0 = blk * BLK
                p_ps = pp.tile([128, BLK, 128], f32)
                for j, (src, roff, coff) in enumerate(_MM_SLICES):
                    xsrc = xa if src == 'XA' else xc
                    if j == 12:  # K=64 single on lo partitions
                        lhsT = wl[0:64, j, :]
                        rhs = xsrc[0:64, r0 + roff:r0 + roff + BLK,
                                   coff:coff + 128]
                    else:
                        lhsT = wl[:, j, :]
                        rhs = xsrc[:, r0 + roff:r0 + roff + BLK, coff:coff + 128]
                    nc.tensor.matmul(p_ps[:], lhsT, rhs,
                                     start=(j == 0), stop=(j == 12))
                psq = work.tile([128, BLK, 128], f32r, tag="psq")
                nc.scalar.activation(psq[:], p_ps[:], AF.Square)
                p_sb = work.tile([128, BLK, 128], f32, tag="p_sb")
                nc.scalar.activation(p_sb[:], p_ps[:], AF.Copy, bias=0.0)
                y_ps = py.tile([8, BLK, 128], f32)
                nc.tensor.matmul(y_ps[:], bd[:], psq[:], start=True, stop=True)
                return p_sb, y_ps

            def stage1(blk, y_ps):
                r0 = blk * BLK
                # factor: F = y/((1+y)*sqrt(y_raw+eps)), y = s^2*y_raw
                a_t = small.tile([8, BLK, 128], f32, tag="a")
                nc.scalar.activation(a_t[:], y_ps[:], AF.Sqrt, bias=eps_t[:])
                y_t = small.tile([8, BLK, 128], f32, tag="y")
                nc.vector.tensor_mul(y_t[:], y_ps[:], s2_sb[:, r0:r0 + BLK, :])
                y1_t = small.tile([8, BLK, 128], f32, tag="y1")
                nc.gpsimd.tensor_scalar_add(y1_t[:], y_t[:], 1.0)
                b_t = small.tile([8, BLK, 128], f32, tag="b")
                nc.gpsimd.tensor_mul(b_t[:], a_t[:], y1_t[:])
                r_t = small.tile([8, BLK, 128], f32, tag="r")
                nc.vector.reciprocal_approx_fast(r_t[:], b_t[:])
                F_t = small.tile([8, BLK, 128], f32r, tag="F")
                nc.vector.tensor_mul(F_t[:], y_t[:], r_t[:])
                fe_ps = pf.tile([128, BLK, 128], f32)
                nc.tensor.matmul(fe_ps[:], ex[:], F_t[:], start=True, stop=True)
                return fe_ps

            def stage2(blk, p_sb, fe_ps):
                r0 = blk * BLK
                v_t = work.tile([128, BLK, 128], f32, tag="v")
                nc.vector.tensor_mul(v_t[:], p_sb[:], fe_ps[:])
                nc.sync.dma_start(out=out_v[:, r0:r0 + BLK, :], in_=v_t[:])

            with loop_ctx:
                live = {}
                for blk in range(N_BLKS + 2):
                    if blk < N_BLKS:
                        p_sb, y_ps = stage0(blk)
                        live[blk] = [p_sb, y_ps, None]
                    if 1 <= blk <= N_BLKS:
                        live[blk - 1][2] = stage1(blk - 1, live[blk - 1][1])
                    if 2 <= blk:
                        p_sb_o, _, fe_o = live.pop(blk - 2)
                        stage2(blk - 2, p_sb_o, fe_o)

    nc.compile()
    return nc


def _prep_in_maps(u, W):
    x = u.reshape(N, CIN, H, W_SP)
    wl = _weight_tiles(W).reshape(128, 13 * 128)
    bd = _block_diag()
    ex = np.ascontiguousarray(bd.T)
    in_maps = []
    for core in range(N_CORES):
        n, half = core // 2, core % 2
        XA, XC = _inputs_core(x[n], half)
        in_maps.append({
            "xa": XA.reshape(128, XROWS * XCOLS),
            "xc": XC.reshape(128, XROWS * XCOLS),
            "wl": wl,
            "bd": bd,
            "ex": ex,
            "s2": _s2_map(half),
        })
    return in_maps


def run(u, W, trace=False):
    """Returns (out [N,T1,Z1,H,W] f32, BassKernelResults)."""
    from concourse.bass_utils import run_bass_kernel_spmd

    if "nc" not in _CACHE:
        _CACHE["nc"] = build_nc()
    nc = _CACHE["nc"]
    in_maps = _prep_in_maps(np.asarray(u, np.float32), np.asarray(W, np.float32))
    res = run_bass_kernel_spmd(nc, in_maps, list(range(N_CORES)), trace=trace)
    out = np.empty((N, T1, Z1, H, W_SP), np.float32)
    for core in range(N_CORES):
        n, half = core // 2, core % 2
        o = res.results[core]["out"].reshape(T1, Z1, ROWS, 128)
        out[n, :, :, half * 64:(half + 1) * 64, :] = o
    return out, res


def kernel(u, W):
    out, _ = run(u, W, trace=False)
    return out



# revision 5
# speedup vs baseline: 2.3833x; 2.3833x over previous
"""Trainium2 Bass kernel for nn_CapsuleLayer_9852654977072.

The reference module collapses mathematically: the routing loop's coupling
logits `b` stay zero (faithfully-reproduced bug in the original torch code),
so routing coefficients are a fixed spatial map s(h,w) = 1/(8*cnt(h,w)) where
cnt is the 5x5 box-count inside the image. The whole module is therefore:

    praw = conv2d(u as [N,64,H,W], Wd as [128,64,5,5], pad=2)
    v    = praw * sqrt(u2) / (c + u2)        # u2 = sum_z1 praw^2 (groups of 16)
    out[n,t1,z1,h,w] = v                     # c = 1/s^2 spatial map

(The eps inside the reference's sqrt(n2+1e-9) is negligible at these
magnitudes; n2/sqrt(n2+eps) == sqrt(n2) to ~1e-7 relative.)

Device strategy (8 cores, SPMD): shard (batch n in 0..3) x (row-half in 0..1).
Each core computes all 128 output channels for 64 rows of one image.

Conv: inputs shipped as XA/XC fp16 [128, 68, 132] whose partition halves hold
u shifted by (+0row,+1row) and (+2row+0col,+2row+1col), columns padded by 2.
Per 4-row block, 13 PSUM-accumulated fp16 matmuls (N=512, full PE rate, FWL
eligible: all K=128, tap 13 zero-padded) cover all 25 taps.

Squash: square (ACT, ->fp16) -> block-diag matmul (u2 over z1) -> on 8
partitions: a=sqrt(u2) [ACT], d=u2+c [DVE], r~=1/d [DVE], F=a*r fp16 [DVE]
-> expand matmul -> v = praw * F [DVE, both PSUM operands].
"""

import numpy as np

T0, Z0, T1, Z1, KK, PAD = 4, 16, 8, 16, 5, 2
N, H, W_SP = 4, 128, 128
CIN, COUT = T0 * Z0, T1 * Z1  # 64, 128
N_CORES = 8
ROWS = 64          # output rows per core
XROWS = 68         # input rows incl. halo
XCOLS = 132        # 128 + 2*PAD
BLK = 4            # output rows per block
N_BLKS = ROWS // BLK

# conv matmul j -> (source, row_off, col_off); weights match in _weight_tiles
_MM_SLICES = (
    [('XA', dy + 2, dx + 2) for dy in (-2, 0) for dx in (-2, -1, 0, 1, 2)]
    + [('XC', 2, 0), ('XC', 2, 2), ('XC', 2, 4)]
)

_CACHE = {}


def _weight_tiles(W):
    Wd = W.transpose(1, 0, 2, 3, 4).reshape(COUT, CIN, KK, KK)
    wl = np.zeros((128, 13, 128), np.float32)  # [k, j, m]
    j = 0
    for dy in (-2, 0):
        for dx in (-2, -1, 0, 1, 2):
            wl[0:64, j, :] = Wd[:, :, dy + 2, dx + 2].T
            wl[64:128, j, :] = Wd[:, :, dy + 3, dx + 2].T
            j += 1
    for dx0 in (-2, 0):
        wl[0:64, j, :] = Wd[:, :, 4, dx0 + 2].T
        wl[64:128, j, :] = Wd[:, :, 4, dx0 + 3].T
        j += 1
    wl[0:64, j, :] = Wd[:, :, 4, 4].T  # single tap (2,2); hi partitions stay 0
    return wl.astype(np.float16)


def _inputs_core(x, half):
    """x: [64, H, W] one image channel-major. Returns XA, XC fp16 [128,68,132]."""
    base = half * 64 - 2
    XA = np.zeros((128, XROWS, XCOLS), np.float16)
    XC = np.zeros((128, XROWS, XCOLS), np.float16)

    def fill(dst, roff, c0, c1):
        lo, hi = max(0, -(base + roff)), min(XROWS, H - base - roff)
        dst[:, lo:hi, c0:c1] = x[:, base + roff + lo:base + roff + hi, :]

    fill(XA[0:64], 0, 2, 130)
    fill(XA[64:128], 1, 2, 130)
    fill(XC[0:64], 2, 2, 130)
    fill(XC[64:128], 2, 1, 129)
    return XA, XC


def _c_map(half):
    """c(h,w) = 1/s^2 = (8*cnt_r*cnt_c)^2 for this core's row half."""
    idx = np.arange(H)
    cnt = (np.minimum(idx + 2, H - 1) - np.maximum(idx - 2, 0) + 1).astype(np.float64)
    c = (8.0 * cnt[:, None] * cnt[None, :]) ** 2  # [H, W]
    c = c[half * 64:(half + 1) * 64, :]
    return np.ascontiguousarray(c.astype(np.float32).reshape(1, ROWS * 128))


def _block_diag():
    bd = np.zeros((128, 8), np.float16)
    bd[np.arange(128), np.arange(128) // 16] = 1.0
    return bd


def build_nc():
    import concourse.bass as bass
    import concourse.bacc as bacc
    import concourse.mybir as mybir
    import concourse.tile as tile

    f32 = mybir.dt.float32
    f16 = mybir.dt.float16
    AF = mybir.ActivationFunctionType

    nc = bacc.Bacc(None, target_bir_lowering=False)
    xa_d = nc.dram_tensor("xa", [128, XROWS * XCOLS], f16, kind="ExternalInput")
    xc_d = nc.dram_tensor("xc", [128, XROWS * XCOLS], f16, kind="ExternalInput")
    wl_d = nc.dram_tensor("wl", [128, 13 * 128], f16, kind="ExternalInput")
    bd_d = nc.dram_tensor("bd", [128, 8], f16, kind="ExternalInput")
    ex_d = nc.dram_tensor("ex", [8, 128], f16, kind="ExternalInput")
    cm_d = nc.dram_tensor("cm", [1, ROWS * 128], f32, kind="ExternalInput")
    out_d = nc.dram_tensor("out", [128, ROWS * 128], f32, kind="ExternalOutput")

    with tile.TileContext(nc) as tc:
        with (
            tc.tile_pool(name="consts", bufs=1) as consts,
            tc.tile_pool(name="work", bufs=4) as work,
            tc.tile_pool(name="small", bufs=4) as small,
            tc.tile_pool(name="pp", bufs=4, space="PSUM") as pp,
            tc.tile_pool(name="pf", bufs=2, space="PSUM") as pf,
            tc.tile_pool(name="py", bufs=2, space="PSUM") as py,
        ):
            wl = consts.tile([128, 13, 128], f16)
            nc.sync.dma_start(
                out=wl, in_=wl_d.ap().rearrange("p (j m) -> p j m", m=128))
            bd = consts.tile([128, 8], f16)
            nc.sync.dma_start(out=bd, in_=bd_d.ap())
            ex = consts.tile([8, 128], f16)
            nc.sync.dma_start(out=ex, in_=ex_d.ap())
            cm_sb = consts.tile([8, ROWS, 128], f32)
            cm_ap = cm_d.ap()
            nc.sync.dma_start(
                out=cm_sb,
                in_=bass.AP(tensor=cm_ap.tensor, offset=cm_ap.offset,
                            ap=[[0, 8], [128, ROWS], [1, 128]]))

            xa = consts.tile([128, XROWS, XCOLS], f16)
            xc = consts.tile([128, XROWS, XCOLS], f16)
            xa_src = xa_d.ap().rearrange("p (r c) -> p r c", c=XCOLS)
            xc_src = xc_d.ap().rearrange("p (r c) -> p r c", c=XCOLS)
            for c0 in range(0, XROWS, 17):
                nc.sync.dma_start(
                    out=xa[:, c0:c0 + 17, :], in_=xa_src[:, c0:c0 + 17, :])
                nc.sync.dma_start(
                    out=xc[:, c0:c0 + 17, :], in_=xc_src[:, c0:c0 + 17, :])

            out_v = out_d.ap().rearrange("p (r c) -> p r c", c=128)

            def stage0(blk):
                r0 = blk * BLK
                p_ps = pp.tile([128, BLK, 128], f32)
                for j, (src, roff, coff) in enumerate(_MM_SLICES):
                    xsrc = xa if src == 'XA' else xc
                    nc.tensor.matmul(
                        p_ps[:], wl[:, j, :],
                        xsrc[:, r0 + roff:r0 + roff + BLK, coff:coff + 128],
                        start=(j == 0), stop=(j == 12))
                psq = work.tile([128, BLK, 128], f16, tag="psq")
                nc.scalar.activation(psq[:], p_ps[:], AF.Square)
                p_sb = work.tile([128, BLK, 128], f16, tag="p_sb")
                nc.scalar.activation(p_sb[:], p_ps[:], AF.Copy, bias=0.0)
                y_ps = py.tile([8, BLK, 128], f32)
                nc.tensor.matmul(y_ps[:], bd[:], psq[:], start=True, stop=True)
                return p_sb, y_ps

            def stage1(blk, y_ps):
                r0 = blk * BLK
                # F = sqrt(u2) / (c + u2)
                a_t = small.tile([8, BLK, 128], f32, tag="a")
                nc.scalar.activation(a_t[:], y_ps[:], AF.Sqrt)
                d_t = small.tile([8, BLK, 128], f32, tag="d")
                nc.vector.tensor_add(d_t[:], y_ps[:], cm_sb[:, r0:r0 + BLK, :])
                r_t = small.tile([8, BLK, 128], f32, tag="r")
                nc.vector.reciprocal_approx_fast(r_t[:], d_t[:])
                F_t = small.tile([8, BLK, 128], f16, tag="F")
                nc.vector.tensor_mul(F_t[:], a_t[:], r_t[:])
                fe_ps = pf.tile([128, BLK, 128], f32)
                nc.tensor.matmul(fe_ps[:], ex[:], F_t[:], start=True, stop=True)
                return fe_ps

            def stage2(blk, p_sb, fe_ps):
                r0 = blk * BLK
                v_t = work.tile([128, BLK, 128], f32, tag="v")
                nc.vector.tensor_mul(v_t[:], p_sb[:], fe_ps[:])
                nc.sync.dma_start(out=out_v[:, r0:r0 + BLK, :], in_=v_t[:])

            live = {}
            for blk in range(N_BLKS + 2):
                if blk < N_BLKS:
                    p_sb, y_ps = stage0(blk)
                    live[blk] = [p_sb, y_ps, None]
                if 1 <= blk <= N_BLKS:
                    live[blk - 1][2] = stage1(blk - 1, live[blk - 1][1])
                if 2 <= blk:
                    p_sb_o, _, fe_o = live.pop(blk - 2)
                    stage2(blk - 2, p_sb_o, fe_o)

    nc.compile()
    return nc


def _prep_in_maps(u, W):
    x = u.reshape(N, CIN, H, W_SP)
    wl = _weight_tiles(W).reshape(128, 13 * 128)
    bd = _block_diag()
    ex = np.ascontiguousarray(bd.T)
    in_maps = []
    for core in range(N_CORES):
        n, half = core // 2, core % 2
        XA, XC = _inputs_core(x[n], half)
        in_maps.append({
            "xa": XA.reshape(128, XROWS * XCOLS),
            "xc": XC.reshape(128, XROWS * XCOLS),
            "wl": wl,
            "bd": bd,
            "ex": ex,
            "cm": _c_map(half),
        })
    return in_maps


def run(u, W, trace=False):
    """Returns (out [N,T1,Z1,H,W] f32, BassKernelResults)."""
    from concourse.bass_utils import run_bass_kernel_spmd

    if "nc" not in _CACHE:
        _CACHE["nc"] = build_nc()
    nc = _CACHE["nc"]
    in_maps = _prep_in_maps(np.asarray(u, np.float32), np.asarray(W, np.float32))
    res = run_bass_kernel_spmd(nc, in_maps, list(range(N_CORES)), trace=trace)
    out = np.empty((N, T1, Z1, H, W_SP), np.float32)
    for core in range(N_CORES):
        n, half = core // 2, core % 2
        o = res.results[core]["out"].reshape(T1, Z1, ROWS, 128)
        out[n, :, :, half * 64:(half + 1) * 64, :] = o
    return out, res


def kernel(u, W):
    out, _ = run(u, W, trace=False)
    return out


# revision 10
# speedup vs baseline: 2.3851x; 1.0008x over previous
"""Trainium2 Bass kernel for nn_CapsuleLayer_9852654977072.

The reference module collapses mathematically: the routing loop's coupling
logits `b` stay zero (faithfully-reproduced bug in the original torch code),
so routing coefficients are a fixed spatial map s(h,w) = 1/(8*cnt(h,w)) where
cnt is the 5x5 box-count inside the image. The whole module is therefore:

    praw = conv2d(u as [N,64,H,W], Wd as [128,64,5,5], pad=2)
    v    = praw * sqrt(u2) / (c + u2)        # u2 = sum_z1 praw^2 (groups of 16)
    out[n,t1,z1,h,w] = v                     # c = 1/s^2 spatial map

(The eps inside the reference's sqrt(n2+1e-9) is negligible at these
magnitudes; n2/sqrt(n2+eps) == sqrt(n2) to ~1e-7 relative.)

Device strategy (8 cores, SPMD): shard (batch n in 0..3) x (row-half in 0..1).
Each core computes all 128 output channels for 64 rows of one image.

Conv: inputs shipped as XA/XC fp16 [128, 68, 132] whose partition halves hold
u shifted by (+0row,+1row) and (+2row+0col,+2row+1col), columns padded by 2.
Per 4-row block, 13 PSUM-accumulated fp16 matmuls (N=512, full PE rate, FWL
eligible: all K=128, tap 13 zero-padded) cover all 25 taps.

Squash: square (ACT, ->fp16) -> block-diag matmul (u2 over z1) -> on 8
partitions: a=sqrt(u2) [ACT], d=u2+c [DVE], r~=1/d [DVE], F=a*r fp16 [DVE]
-> expand matmul -> v = praw * F [DVE, both PSUM operands].
"""

import numpy as np

T0, Z0, T1, Z1, KK, PAD = 4, 16, 8, 16, 5, 2
N, H, W_SP = 4, 128, 128
CIN, COUT = T0 * Z0, T1 * Z1  # 64, 128
N_CORES = 8
ROWS = 64          # output rows per core
XROWS = 68         # input rows incl. halo
XCOLS = 132        # 128 + 2*PAD
BLK = 4            # output rows per block
N_BLKS = ROWS // BLK

# conv matmul j -> (source, row_off, col_off); weights match in _weight_tiles
_MM_SLICES = (
    [('XA', dy + 2, dx + 2) for dy in (-2, 0) for dx in (-2, -1, 0, 1, 2)]
    + [('XC', 2, 0), ('XC', 2, 2), ('XC', 2, 4)]
)

_CACHE = {}


def _weight_tiles(W):
    Wd = W.transpose(1, 0, 2, 3, 4).reshape(COUT, CIN, KK, KK)
    wl = np.zeros((128, 13, 128), np.float32)  # [k, j, m]
    j = 0
    for dy in (-2, 0):
        for dx in (-2, -1, 0, 1, 2):
            wl[0:64, j, :] = Wd[:, :, dy + 2, dx + 2].T
            wl[64:128, j, :] = Wd[:, :, dy + 3, dx + 2].T
            j += 1
    for dx0 in (-2, 0):
        wl[0:64, j, :] = Wd[:, :, 4, dx0 + 2].T
        wl[64:128, j, :] = Wd[:, :, 4, dx0 + 3].T
        j += 1
    wl[0:64, j, :] = Wd[:, :, 4, 4].T  # single tap (2,2); hi partitions stay 0
    return wl.astype(np.float16)


def _inputs_core(x, half):
    """x: [64, H, W] one image channel-major. Returns XA, XC fp16 [128,68,132]."""
    base = half * 64 - 2
    XA = np.zeros((128, XROWS, XCOLS), np.float16)
    XC = np.zeros((128, XROWS, XCOLS), np.float16)

    def fill(dst, roff, c0, c1):
        lo, hi = max(0, -(base + roff)), min(XROWS, H - base - roff)
        dst[:, lo:hi, c0:c1] = x[:, base + roff + lo:base + roff + hi, :]

    fill(XA[0:64], 0, 2, 130)
    fill(XA[64:128], 1, 2, 130)
    fill(XC[0:64], 2, 2, 130)
    fill(XC[64:128], 2, 1, 129)
    return XA, XC


def _c_map(half):
    """c(h,w) = 1/s^2 = (8*cnt_r*cnt_c)^2 for this core's row half."""
    idx = np.arange(H)
    cnt = (np.minimum(idx + 2, H - 1) - np.maximum(idx - 2, 0) + 1).astype(np.float64)
    c = (8.0 * cnt[:, None] * cnt[None, :]) ** 2  # [H, W]
    c = c[half * 64:(half + 1) * 64, :]
    return np.ascontiguousarray(c.astype(np.float32).reshape(1, ROWS * 128))


def _block_diag():
    bd = np.zeros((128, 8), np.float16)
    bd[np.arange(128), np.arange(128) // 16] = 1.0
    return bd


def build_nc():
    import concourse.bass as bass
    import concourse.bacc as bacc
    import concourse.mybir as mybir
    import concourse.tile as tile

    f32 = mybir.dt.float32
    f16 = mybir.dt.float16
    AF = mybir.ActivationFunctionType

    nc = bacc.Bacc(None, target_bir_lowering=False)
    xa_d = nc.dram_tensor("xa", [128, XROWS * XCOLS], f16, kind="ExternalInput")
    xc_d = nc.dram_tensor("xc", [128, XROWS * XCOLS], f16, kind="ExternalInput")
    wl_d = nc.dram_tensor("wl", [128, 13 * 128], f16, kind="ExternalInput")
    bd_d = nc.dram_tensor("bd", [128, 8], f16, kind="ExternalInput")
    ex_d = nc.dram_tensor("ex", [8, 128], f16, kind="ExternalInput")
    cm_d = nc.dram_tensor("cm", [1, ROWS * 128], f32, kind="ExternalInput")
    out_d = nc.dram_tensor("out", [128, ROWS * 128], f32, kind="ExternalOutput")

    with tile.TileContext(nc) as tc:
        with (
            tc.tile_pool(name="consts", bufs=1) as consts,
            tc.tile_pool(name="work", bufs=4) as work,
            tc.tile_pool(name="small", bufs=4) as small,
            tc.tile_pool(name="pp", bufs=3, space="PSUM") as pp,
            tc.tile_pool(name="pf", bufs=2, space="PSUM") as pf,
            tc.tile_pool(name="py", bufs=3, space="PSUM") as py,
        ):
            wl = consts.tile([128, 13, 128], f16)
            nc.sync.dma_start(
                out=wl, in_=wl_d.ap().rearrange("p (j m) -> p j m", m=128))
            bd = consts.tile([128, 8], f16)
            nc.sync.dma_start(out=bd, in_=bd_d.ap())
            ex = consts.tile([8, 128], f16)
            nc.sync.dma_start(out=ex, in_=ex_d.ap())
            cm_sb = consts.tile([8, ROWS, 128], f32)
            cm_ap = cm_d.ap()
            nc.sync.dma_start(
                out=cm_sb,
                in_=bass.AP(tensor=cm_ap.tensor, offset=cm_ap.offset,
                            ap=[[0, 8], [128, ROWS], [1, 128]]))

            xa = consts.tile([128, XROWS, XCOLS], f16)
            xc = consts.tile([128, XROWS, XCOLS], f16)
            xa_src = xa_d.ap().rearrange("p (r c) -> p r c", c=XCOLS)
            xc_src = xc_d.ap().rearrange("p (r c) -> p r c", c=XCOLS)
            for c0 in range(0, XROWS, 17):
                nc.sync.dma_start(
                    out=xa[:, c0:c0 + 17, :], in_=xa_src[:, c0:c0 + 17, :])
                nc.sync.dma_start(
                    out=xc[:, c0:c0 + 17, :], in_=xc_src[:, c0:c0 + 17, :])

            out_v = out_d.ap().rearrange("p (r c) -> p r c", c=128)

            # HAM warm-up: run dummy matmuls during the input-DMA fill so the
            # PE clock gate opens before the first real conv block.
            wtmp = consts.tile([128, 128], f16)
            nc.vector.memset(wtmp[:], 0.125)
            pwarm = pp.tile([128, BLK, 128], f32, tag="p_ps")
            for _ in range(20):
                nc.tensor.matmul(pwarm[:, 0, :], wtmp[:], wtmp[:],
                                 start=True, stop=True)

            def stage0(blk):
                r0 = blk * BLK
                p_ps = pp.tile([128, BLK, 128], f32)
                for j, (src, roff, coff) in enumerate(_MM_SLICES):
                    xsrc = xa if src == 'XA' else xc
                    nc.tensor.matmul(
                        p_ps[:], wl[:, j, :],
                        xsrc[:, r0 + roff:r0 + roff + BLK, coff:coff + 128],
                        start=(j == 0), stop=(j == 12))
                psq = work.tile([128, BLK, 128], f16, tag="psq")
                nc.scalar.activation(psq[:], p_ps[:], AF.Square)
                p_sb = work.tile([128, BLK, 128], f16, tag="p_sb")
                nc.scalar.activation(p_sb[:], p_ps[:], AF.Copy, bias=0.0)
                y_ps = py.tile([8, BLK, 128], f32)
                nc.tensor.matmul(y_ps[:], bd[:], psq[:], start=True, stop=True)
                return p_sb, y_ps

            def stage1a(blk, y_ps):
                # issued FIRST in the step so ACT unblocks the DVE chain early
                a_t = small.tile([8, BLK, 128], f32, tag="a")
                nc.scalar.activation(a_t[:], y_ps[:], AF.Sqrt)
                return a_t

            def stage1b(blk, y_ps, a_t):
                r0 = blk * BLK
                # F = sqrt(u2) / (c + u2)
                d_t = small.tile([8, BLK, 128], f32, tag="d")
                nc.vector.tensor_add(d_t[:], y_ps[:], cm_sb[:, r0:r0 + BLK, :])
                r_t = small.tile([8, BLK, 128], f32, tag="r")
                nc.vector.reciprocal_approx_fast(r_t[:], d_t[:])
                F_t = small.tile([8, BLK, 128], f16, tag="F")
                nc.vector.tensor_mul(F_t[:], a_t[:], r_t[:])
                fe_ps = pf.tile([128, BLK, 128], f32)
                nc.tensor.matmul(fe_ps[:], ex[:], F_t[:], start=True, stop=True)
                return fe_ps

            def stage2(blk, p_sb, fe_ps):
                r0 = blk * BLK
                v_t = work.tile([128, BLK, 128], f32, tag="v")
                nc.vector.tensor_mul(v_t[:], p_sb[:], fe_ps[:])
                nc.sync.dma_start(out=out_v[:, r0:r0 + BLK, :], in_=v_t[:])

            live = {}
            for blk in range(N_BLKS + 2):
                if 1 <= blk <= N_BLKS:
                    live[blk - 1][2] = stage1a(blk - 1, live[blk - 1][1])
                if blk < N_BLKS:
                    p_sb, y_ps = stage0(blk)
                    live[blk] = [p_sb, y_ps, None, None]
                if 1 <= blk <= N_BLKS:
                    live[blk - 1][3] = stage1b(
                        blk - 1, live[blk - 1][1], live[blk - 1][2])
                if 2 <= blk:
                    p_sb_o, _, _, fe_o = live.pop(blk - 2)
                    stage2(blk - 2, p_sb_o, fe_o)

    nc.compile()
    return nc


def _prep_in_maps(u, W):
    x = u.reshape(N, CIN, H, W_SP)
    wl = _weight_tiles(W).reshape(128, 13 * 128)
    bd = _block_diag()
    ex = np.ascontiguousarray(bd.T)
    in_maps = []
    for core in range(N_CORES):
        n, half = core // 2, core % 2
        XA, XC = _inputs_core(x[n], half)
        in_maps.append({
            "xa": XA.reshape(128, XROWS * XCOLS),
            "xc": XC.reshape(128, XROWS * XCOLS),
            "wl": wl,
            "bd": bd,
            "ex": ex,
            "cm": _c_map(half),
        })
    return in_maps


def run(u, W, trace=False):
    """Returns (out [N,T1,Z1,H,W] f32, BassKernelResults)."""
    from concourse.bass_utils import run_bass_kernel_spmd

    if "nc" not in _CACHE:
        _CACHE["nc"] = build_nc()
    nc = _CACHE["nc"]
    in_maps = _prep_in_maps(np.asarray(u, np.float32), np.asarray(W, np.float32))
    res = run_bass_kernel_spmd(nc, in_maps, list(range(N_CORES)), trace=trace)
    out = np.empty((N, T1, Z1, H, W_SP), np.float32)
    for core in range(N_CORES):
        n, half = core // 2, core % 2
        o = res.results[core]["out"].reshape(T1, Z1, ROWS, 128)
        out[n, :, :, half * 64:(half + 1) * 64, :] = o
    return out, res


def kernel(u, W):
    out, _ = run(u, W, trace=False)
    return out


# revision 12
# speedup vs baseline: 2.4711x; 1.0361x over previous
"""Trainium2 Bass kernel for nn_CapsuleLayer_9852654977072.

The reference module collapses mathematically: the routing loop's coupling
logits `b` stay zero (faithfully-reproduced bug in the original torch code),
so routing coefficients are a fixed spatial map s(h,w) = 1/(8*cnt(h,w)) where
cnt is the 5x5 box-count inside the image. The whole module is therefore:

    praw = conv2d(u as [N,64,H,W], Wd as [128,64,5,5], pad=2)
    v    = praw * sqrt(u2) / (c + u2)        # u2 = sum_z1 praw^2 (groups of 16)
    out[n,t1,z1,h,w] = v                     # c = 1/s^2 spatial map

(The eps inside the reference's sqrt(n2+1e-9) is negligible at these
magnitudes; n2/sqrt(n2+eps) == sqrt(n2) to ~1e-7 relative.)

Device strategy (8 cores, SPMD): shard (batch n in 0..3) x (row-half in 0..1).
Each core computes all 128 output channels for 64 rows of one image.

Conv: inputs shipped as XA/XC fp16 [128, 68, 132] whose partition halves hold
u shifted by (+0row,+1row) and (+2row+0col,+2row+1col), columns padded by 2.
Per 4-row block, 13 PSUM-accumulated fp16 matmuls (N=512, full PE rate, FWL
eligible: all K=128, tap 13 zero-padded) cover all 25 taps.

Squash: square (ACT, ->fp16) -> block-diag matmul (u2 over z1) -> on 8
partitions: a=sqrt(u2) [ACT], d=u2+c [DVE], r~=1/d [DVE], F=a*r fp16 [DVE]
-> expand matmul -> v = praw * F [DVE, both PSUM operands].
"""

import numpy as np

T0, Z0, T1, Z1, KK, PAD = 4, 16, 8, 16, 5, 2
N, H, W_SP = 4, 128, 128
CIN, COUT = T0 * Z0, T1 * Z1  # 64, 128
N_CORES = 8
ROWS = 64          # output rows per core
XROWS = 68         # input rows incl. halo
XCOLS = 132        # 128 + 2*PAD
BLK = 4            # output rows per block
N_BLKS = ROWS // BLK

# conv matmul j -> (source, row_off, col_off); weights match in _weight_tiles
_MM_SLICES = (
    [('XA', dy + 2, dx + 2) for dy in (-2, 0) for dx in (-2, -1, 0, 1, 2)]
    + [('XC', 2, 0), ('XC', 2, 2), ('XC', 2, 4)]
)

_CACHE = {}


def _weight_tiles(W):
    Wd = W.transpose(1, 0, 2, 3, 4).reshape(COUT, CIN, KK, KK)
    wl = np.zeros((128, 13, 128), np.float32)  # [k, j, m]
    j = 0
    for dy in (-2, 0):
        for dx in (-2, -1, 0, 1, 2):
            wl[0:64, j, :] = Wd[:, :, dy + 2, dx + 2].T
            wl[64:128, j, :] = Wd[:, :, dy + 3, dx + 2].T
            j += 1
    for dx0 in (-2, 0):
        wl[0:64, j, :] = Wd[:, :, 4, dx0 + 2].T
        wl[64:128, j, :] = Wd[:, :, 4, dx0 + 3].T
        j += 1
    wl[0:64, j, :] = Wd[:, :, 4, 4].T  # single tap (2,2); hi partitions stay 0
    return wl.astype(np.float16)


def _inputs_core(x, half):
    """x: [64, H, W] one image channel-major. Returns XA, XC fp16 [128,68,132]."""
    base = half * 64 - 2
    XA = np.zeros((128, XROWS, XCOLS), np.float16)
    XC = np.zeros((128, XROWS, XCOLS), np.float16)

    def fill(dst, roff, c0, c1):
        lo, hi = max(0, -(base + roff)), min(XROWS, H - base - roff)
        dst[:, lo:hi, c0:c1] = x[:, base + roff + lo:base + roff + hi, :]

    fill(XA[0:64], 0, 2, 130)
    fill(XA[64:128], 1, 2, 130)
    fill(XC[0:64], 2, 2, 130)
    fill(XC[64:128], 2, 1, 129)
    return XA, XC


def _c_map(half):
    """c(h,w) = 1/s^2 = (8*cnt_r*cnt_c)^2 for this core's row half."""
    idx = np.arange(H)
    cnt = (np.minimum(idx + 2, H - 1) - np.maximum(idx - 2, 0) + 1).astype(np.float64)
    c = (8.0 * cnt[:, None] * cnt[None, :]) ** 2  # [H, W]
    c = c[half * 64:(half + 1) * 64, :]
    return np.ascontiguousarray(c.astype(np.float32).reshape(1, ROWS * 128))


def _block_diag():
    bd = np.zeros((128, 8), np.float16)
    bd[np.arange(128), np.arange(128) // 16] = 1.0
    return bd


def build_nc():
    import concourse.bass as bass
    import concourse.bacc as bacc
    import concourse.mybir as mybir
    import concourse.tile as tile

    f32 = mybir.dt.float32
    f16 = mybir.dt.float16
    AF = mybir.ActivationFunctionType

    nc = bacc.Bacc(None, target_bir_lowering=False)
    xa_d = nc.dram_tensor("xa", [128, XROWS * XCOLS], f16, kind="ExternalInput")
    xc_d = nc.dram_tensor("xc", [128, XROWS * XCOLS], f16, kind="ExternalInput")
    wl_d = nc.dram_tensor("wl", [128, 13 * 128], f16, kind="ExternalInput")
    bd_d = nc.dram_tensor("bd", [128, 8], f16, kind="ExternalInput")
    ex_d = nc.dram_tensor("ex", [8, 128], f16, kind="ExternalInput")
    cm_d = nc.dram_tensor("cm", [1, ROWS * 128], f32, kind="ExternalInput")
    out_d = nc.dram_tensor("out", [128, ROWS * 128], f32, kind="ExternalOutput")

    with tile.TileContext(nc) as tc:
        with (
            tc.tile_pool(name="consts", bufs=1) as consts,
            tc.tile_pool(name="work", bufs=4) as work,
            tc.tile_pool(name="small", bufs=4) as small,
            tc.tile_pool(name="pp", bufs=3, space="PSUM") as pp,
            tc.tile_pool(name="pf", bufs=2, space="PSUM") as pf,
            tc.tile_pool(name="py", bufs=3, space="PSUM") as py,
        ):
            # Input lives in 4 overlapping 20-row tiles per tensor so each
            # conv block depends on exactly ONE tile: block b (tile b//4)
            # reads tile-local rows 4*(b%4) .. 4*(b%4)+5.  This lets block 0
            # start as soon as ~1.4 MB has landed instead of the full 4.6 MB.
            TROWS = 20
            xa_src = xa_d.ap().rearrange("p (r c) -> p r c", c=XCOLS)
            xc_src = xc_d.ap().rearrange("p (r c) -> p r c", c=XCOLS)

            wl = consts.tile([128, 13, 128], f16)
            nc.sync.dma_start(
                out=wl, in_=wl_d.ap().rearrange("p (j m) -> p j m", m=128))
            xa_t = []
            xc_t = []
            for k in range(4):
                xa_t.append(consts.tile([128, TROWS, XCOLS], f16,
                                        name=f"xa{k}", tag=f"xa{k}"))
                xc_t.append(consts.tile([128, TROWS, XCOLS], f16,
                                        name=f"xc{k}", tag=f"xc{k}"))
            nc.sync.dma_start(out=xa_t[0], in_=xa_src[:, 0:TROWS, :])
            nc.sync.dma_start(out=xc_t[0], in_=xc_src[:, 0:TROWS, :])
            bd = consts.tile([128, 8], f16)
            nc.sync.dma_start(out=bd, in_=bd_d.ap())
            ex = consts.tile([8, 128], f16)
            nc.sync.dma_start(out=ex, in_=ex_d.ap())
            cm_sb = consts.tile([8, ROWS, 128], f32)
            cm_ap = cm_d.ap()
            nc.sync.dma_start(
                out=cm_sb,
                in_=bass.AP(tensor=cm_ap.tensor, offset=cm_ap.offset,
                            ap=[[0, 8], [128, ROWS], [1, 128]]))
            for k in range(1, 4):
                r0 = 16 * k
                nc.sync.dma_start(out=xa_t[k], in_=xa_src[:, r0:r0 + TROWS, :])
                nc.sync.dma_start(out=xc_t[k], in_=xc_src[:, r0:r0 + TROWS, :])

            out_v = out_d.ap().rearrange("p (r c) -> p r c", c=128)

            # HAM warm-up: run dummy matmuls during the input-DMA fill so the
            # PE clock gate opens before the first real conv block.
            wtmp = consts.tile([128, 256], f16)
            nc.vector.memset(wtmp[:], 0.125)
            pwarm = pp.tile([128, BLK, 128], f32, tag="p_ps")
            for _ in range(10):
                nc.tensor.matmul(pwarm[:, 0:2, :], wtmp[:, 0:128], wtmp[:],
                                 start=True, stop=True)

            # software pipeline, per step b:
            #   PE : conv(b) x13, bd(b-1), ex(b-2)    <- dense, no waits
            #   ACT: sqrt(b-2), sq(b-1), cp(b-1)
            #   DVE: add(b-2), recip(b-2), F(b-2), v(b-3)
            st = {}  # blk -> [p_ps, psq, p_sb, y_ps, a, F, fe]

            def conv(blk):
                lr0 = (blk % 4) * BLK
                xak, xck = xa_t[blk // 4], xc_t[blk // 4]
                p_ps = pp.tile([128, BLK, 128], f32, tag="p_ps")
                for j, (src, roff, coff) in enumerate(_MM_SLICES):
                    xsrc = xak if src == 'XA' else xck
                    nc.tensor.matmul(
                        p_ps[:], wl[:, j, :],
                        xsrc[:, lr0 + roff:lr0 + roff + BLK, coff:coff + 128],
                        start=(j == 0), stop=(j == 12))
                st[blk] = {"p": p_ps}

            def act_sq(blk):
                s = st[blk]
                psq = work.tile([128, BLK, 128], f16, tag="psq")
                nc.scalar.activation(psq[:], s["p"][:], AF.Square)
                p_sb = work.tile([128, BLK, 128], f16, tag="p_sb")
                nc.scalar.activation(p_sb[:], s["p"][:], AF.Copy, bias=0.0)
                s["psq"], s["p_sb"] = psq, p_sb

            def pe_bd(blk):
                s = st[blk]
                y_ps = py.tile([8, BLK, 128], f32)
                nc.tensor.matmul(y_ps[:], bd[:], s["psq"][:],
                                 start=True, stop=True)
                s["y"] = y_ps

            def act_sqrt(blk):
                s = st[blk]
                a_t = small.tile([8, BLK, 128], f32, tag="a")
                nc.scalar.activation(a_t[:], s["y"][:], AF.Sqrt)
                s["a"] = a_t

            def dve_chain(blk):
                s = st[blk]
                r0 = blk * BLK
                # F = sqrt(u2) / (c + u2)
                d_t = small.tile([8, BLK, 128], f32, tag="d")
                nc.vector.tensor_add(d_t[:], s["y"][:], cm_sb[:, r0:r0 + BLK, :])
                r_t = small.tile([8, BLK, 128], f32, tag="r")
                nc.vector.reciprocal_approx_fast(r_t[:], d_t[:])
                F_t = small.tile([8, BLK, 128], f16, tag="F")
                nc.vector.tensor_mul(F_t[:], s["a"][:], r_t[:])
                s["F"] = F_t

            def pe_ex(blk):
                s = st[blk]
                fe_ps = pf.tile([128, BLK, 128], f32)
                nc.tensor.matmul(fe_ps[:], ex[:], s["F"][:],
                                 start=True, stop=True)
                s["fe"] = fe_ps

            def dve_out(blk):
                s = st.pop(blk)
                r0 = blk * BLK
                v_t = work.tile([128, BLK, 128], f32, tag="v")
                nc.vector.tensor_mul(v_t[:], s["p_sb"][:], s["fe"][:])
                nc.sync.dma_start(out=out_v[:, r0:r0 + BLK, :], in_=v_t[:])

            for b in range(N_BLKS + 3):
                if 2 <= b <= N_BLKS + 1:
                    act_sqrt(b - 2)
                if b < N_BLKS:
                    conv(b)
                if 1 <= b <= N_BLKS:
                    act_sq(b - 1)
                    pe_bd(b - 1)
                if 2 <= b <= N_BLKS + 1:
                    dve_chain(b - 2)
                    pe_ex(b - 2)
                if 3 <= b:
                    dve_out(b - 3)

    nc.compile()
    return nc


def _prep_in_maps(u, W):
    x = u.reshape(N, CIN, H, W_SP)
    wl = _weight_tiles(W).reshape(128, 13 * 128)
    bd = _block_diag()
    ex = np.ascontiguousarray(bd.T)
    in_maps = []
    for core in range(N_CORES):
        n, half = core // 2, core % 2
        XA, XC = _inputs_core(x[n], half)
        in_maps.append({
            "xa": XA.reshape(128, XROWS * XCOLS),
            "xc": XC.reshape(128, XROWS * XCOLS),
            "wl": wl,
            "bd": bd,
            "ex": ex,
            "cm": _c_map(half),
        })
    return in_maps


def run(u, W, trace=False):
    """Returns (out [N,T1,Z1,H,W] f32, BassKernelResults)."""
    from concourse.bass_utils import run_bass_kernel_spmd

    if "nc" not in _CACHE:
        _CACHE["nc"] = build_nc()
    nc = _CACHE["nc"]
    in_maps = _prep_in_maps(np.asarray(u, np.float32), np.asarray(W, np.float32))
    res = run_bass_kernel_spmd(nc, in_maps, list(range(N_CORES)), trace=trace)
    out = np.empty((N, T1, Z1, H, W_SP), np.float32)
    for core in range(N_CORES):
        n, half = core // 2, core % 2
        o = res.results[core]["out"].reshape(T1, Z1, ROWS, 128)
        out[n, :, :, half * 64:(half + 1) * 64, :] = o
    return out, res


def kernel(u, W):
    out, _ = run(u, W, trace=False)
    return out


# revision 17
# speedup vs baseline: 2.4957x; 1.0100x over previous
"""Trainium2 Bass kernel for nn_CapsuleLayer_9852654977072.

The reference module collapses mathematically: the routing loop's coupling
logits `b` stay zero (faithfully-reproduced bug in the original torch code),
so routing coefficients are a fixed spatial map s(h,w) = 1/(8*cnt(h,w)) where
cnt is the 5x5 box-count inside the image. The whole module is therefore:

    praw = conv2d(u as [N,64,H,W], Wd as [128,64,5,5], pad=2)
    v    = praw * sqrt(u2) / (c + u2)        # u2 = sum_z1 praw^2 (groups of 16)
    out[n,t1,z1,h,w] = v                     # c = 1/s^2 spatial map

(The eps inside the reference's sqrt(n2+1e-9) is negligible at these
magnitudes; n2/sqrt(n2+eps) == sqrt(n2) to ~1e-7 relative.)

Device strategy (8 cores, SPMD): shard (batch n in 0..3) x (row-half in 0..1).
Each core computes all 128 output channels for 64 rows of one image.

Conv: inputs shipped as XA/XC fp16 [128, 68, 132] whose partition halves hold
u shifted by (+0row,+1row) and (+2row+0col,+2row+1col), columns padded by 2.
Per 4-row block, 13 PSUM-accumulated fp16 matmuls (N=512, full PE rate, FWL
eligible: all K=128, tap 13 zero-padded) cover all 25 taps.

Squash: square (ACT, ->fp16) -> block-diag matmul (u2 over z1) -> on 8
partitions: a=sqrt(u2) [ACT], d=u2+c [DVE], r~=1/d [DVE], F=a*r fp16 [DVE]
-> expand matmul -> v = praw * F [DVE, both PSUM operands].
"""

import numpy as np

T0, Z0, T1, Z1, KK, PAD = 4, 16, 8, 16, 5, 2
N, H, W_SP = 4, 128, 128
CIN, COUT = T0 * Z0, T1 * Z1  # 64, 128
N_CORES = 8
ROWS = 64          # output rows per core
XROWS = 68         # input rows incl. halo
XCOLS = 132        # 128 + 2*PAD
BLK = 4            # output rows per block
N_BLKS = ROWS // BLK

# conv matmul j -> (source, row_off, col_off); weights match in _weight_tiles
_MM_SLICES = (
    [('XA', dy + 2, dx + 2) for dy in (-2, 0) for dx in (-2, -1, 0, 1, 2)]
    + [('XC', 2, 0), ('XC', 2, 2), ('XC', 2, 4)]
)

_CACHE = {}


def _weight_tiles(W):
    Wd = W.transpose(1, 0, 2, 3, 4).reshape(COUT, CIN, KK, KK)
    wl = np.zeros((128, 13, 128), np.float32)  # [k, j, m]
    j = 0
    for dy in (-2, 0):
        for dx in (-2, -1, 0, 1, 2):
            wl[0:64, j, :] = Wd[:, :, dy + 2, dx + 2].T
            wl[64:128, j, :] = Wd[:, :, dy + 3, dx + 2].T
            j += 1
    for dx0 in (-2, 0):
        wl[0:64, j, :] = Wd[:, :, 4, dx0 + 2].T
        wl[64:128, j, :] = Wd[:, :, 4, dx0 + 3].T
        j += 1
    wl[0:64, j, :] = Wd[:, :, 4, 4].T  # single tap (2,2); hi partitions stay 0
    return wl.astype(np.float16)


def _inputs_core(x, half):
    """x: [64, H, W] one image channel-major. Returns XA, XC fp16 [128,68,132]."""
    base = half * 64 - 2
    XA = np.zeros((128, XROWS, XCOLS), np.float16)
    XC = np.zeros((128, XROWS, XCOLS), np.float16)

    def fill(dst, roff, c0, c1):
        lo, hi = max(0, -(base + roff)), min(XROWS, H - base - roff)
        dst[:, lo:hi, c0:c1] = x[:, base + roff + lo:base + roff + hi, :]

    fill(XA[0:64], 0, 2, 130)
    fill(XA[64:128], 1, 2, 130)
    fill(XC[0:64], 2, 2, 130)
    fill(XC[64:128], 2, 1, 129)
    return XA, XC


def _c_map(half):
    """c(h,w) = 1/s^2 = (8*cnt_r*cnt_c)^2 for this core's row half."""
    idx = np.arange(H)
    cnt = (np.minimum(idx + 2, H - 1) - np.maximum(idx - 2, 0) + 1).astype(np.float64)
    c = (8.0 * cnt[:, None] * cnt[None, :]) ** 2  # [H, W]
    c = c[half * 64:(half + 1) * 64, :]
    return np.ascontiguousarray(c.astype(np.float32).reshape(1, ROWS * 128))


def _block_diag():
    bd = np.zeros((128, 8), np.float16)
    bd[np.arange(128), np.arange(128) // 16] = 1.0
    return bd


def build_nc():
    import concourse.bass as bass
    import concourse.bacc as bacc
    import concourse.mybir as mybir
    import concourse.tile as tile

    f32 = mybir.dt.float32
    f16 = mybir.dt.float16
    AF = mybir.ActivationFunctionType

    nc = bacc.Bacc(None, target_bir_lowering=False)
    xa_d = nc.dram_tensor("xa", [128, XROWS * XCOLS], f16, kind="ExternalInput")
    xc_d = nc.dram_tensor("xc", [128, XROWS * XCOLS], f16, kind="ExternalInput")
    wl_d = nc.dram_tensor("wl", [128, 13 * 128], f16, kind="ExternalInput")
    bd_d = nc.dram_tensor("bd", [128, 8], f16, kind="ExternalInput")
    ex_d = nc.dram_tensor("ex", [8, 128], f16, kind="ExternalInput")
    cm_d = nc.dram_tensor("cm", [1, ROWS * 128], f32, kind="ExternalInput")
    out_d = nc.dram_tensor("out", [128, ROWS * 128], f16, kind="ExternalOutput")

    with tile.TileContext(nc) as tc:
        with (
            tc.tile_pool(name="consts", bufs=1) as consts,
            tc.tile_pool(name="work", bufs=4) as work,
            tc.tile_pool(name="small", bufs=4) as small,
            tc.tile_pool(name="pp", bufs=3, space="PSUM") as pp,
            tc.tile_pool(name="pf", bufs=2, space="PSUM") as pf,
            tc.tile_pool(name="py", bufs=3, space="PSUM") as py,
        ):
            # Input lives in 4 overlapping 20-row tiles per tensor so each
            # conv block depends on exactly ONE tile: block b (tile b//4)
            # reads tile-local rows 4*(b%4) .. 4*(b%4)+5.  This lets block 0
            # start as soon as ~1.4 MB has landed instead of the full 4.6 MB.
            TROWS = 20
            xa_src = xa_d.ap().rearrange("p (r c) -> p r c", c=XCOLS)
            xc_src = xc_d.ap().rearrange("p (r c) -> p r c", c=XCOLS)

            wl = consts.tile([128, 13, 128], f16)
            nc.sync.dma_start(
                out=wl, in_=wl_d.ap().rearrange("p (j m) -> p j m", m=128))
            xa_t = []
            xc_t = []
            for k in range(4):
                xa_t.append(consts.tile([128, TROWS, XCOLS], f16,
                                        name=f"xa{k}", tag=f"xa{k}"))
                xc_t.append(consts.tile([128, TROWS, XCOLS], f16,
                                        name=f"xc{k}", tag=f"xc{k}"))
            nc.sync.dma_start(out=xa_t[0], in_=xa_src[:, 0:TROWS, :])
            nc.sync.dma_start(out=xc_t[0], in_=xc_src[:, 0:TROWS, :])
            bd = consts.tile([128, 8], f16)
            nc.sync.dma_start(out=bd, in_=bd_d.ap())
            ex = consts.tile([8, 128], f16)
            nc.sync.dma_start(out=ex, in_=ex_d.ap())
            cm_sb = consts.tile([8, ROWS, 128], f32)
            cm_ap = cm_d.ap()
            nc.sync.dma_start(
                out=cm_sb,
                in_=bass.AP(tensor=cm_ap.tensor, offset=cm_ap.offset,
                            ap=[[0, 8], [128, ROWS], [1, 128]]))
            def load_tile(k):
                # issued from inside the block loop: late tiles must not
                # steal round-robin DMA bandwidth from tile 0 at startup
                r0 = 16 * k
                nc.sync.dma_start(out=xa_t[k], in_=xa_src[:, r0:r0 + TROWS, :])
                nc.sync.dma_start(out=xc_t[k], in_=xc_src[:, r0:r0 + TROWS, :])

            out_v = out_d.ap().rearrange("p (r c) -> p r c", c=128)

            # HAM warm-up: run dummy matmuls during the input-DMA fill so the
            # PE clock gate opens before the first real conv block.
            wtmp = consts.tile([128, 256], f16)
            nc.vector.memset(wtmp[:], 0.125)
            pwarm = pp.tile([128, BLK, 128], f32, tag="p_ps")
            for _ in range(14):
                nc.tensor.matmul(pwarm[:, 0:2, :], wtmp[:, 0:128], wtmp[:],
                                 start=True, stop=True)

            # software pipeline, per step b:
            #   PE : conv(b) x13, bd(b-1), ex(b-2)    <- dense, no waits
            #   ACT: sqrt(b-2), sq(b-1), cp(b-1)
            #   DVE: add(b-2), recip(b-2), F(b-2), v(b-3)
            st = {}  # blk -> [p_ps, psq, p_sb, y_ps, a, F, fe]

            def conv(blk):
                lr0 = (blk % 4) * BLK
                xak, xck = xa_t[blk // 4], xc_t[blk // 4]
                p_ps = pp.tile([128, BLK, 128], f32, tag="p_ps")
                for j, (src, roff, coff) in enumerate(_MM_SLICES):
                    xsrc = xak if src == 'XA' else xck
                    nc.tensor.matmul(
                        p_ps[:], wl[:, j, :],
                        xsrc[:, lr0 + roff:lr0 + roff + BLK, coff:coff + 128],
                        start=(j == 0), stop=(j == 12))
                st[blk] = {"p": p_ps}

            def act_sq(blk):
                s = st[blk]
                psq = work.tile([128, BLK, 128], f16, tag="psq")
                nc.scalar.activation(psq[:], s["p"][:], AF.Square)
                p_sb = work.tile([128, BLK, 128], f16, tag="p_sb")
                nc.scalar.activation(p_sb[:], s["p"][:], AF.Copy, bias=0.0)
                s["psq"], s["p_sb"] = psq, p_sb

            def pe_bd(blk):
                s = st[blk]
                y_ps = py.tile([8, BLK, 128], f32)
                nc.tensor.matmul(y_ps[:], bd[:], s["psq"][:],
                                 start=True, stop=True)
                s["y"] = y_ps

            def act_sqrt(blk):
                s = st[blk]
                a_t = small.tile([8, BLK, 128], f32, tag="a")
                nc.scalar.activation(a_t[:], s["y"][:], AF.Sqrt)
                s["a"] = a_t

            def dve_chain(blk):
                s = st[blk]
                r0 = blk * BLK
                # F = sqrt(u2) / (c + u2)
                d_t = small.tile([8, BLK, 128], f32, tag="d")
                nc.vector.tensor_add(d_t[:], s["y"][:], cm_sb[:, r0:r0 + BLK, :])
                r_t = small.tile([8, BLK, 128], f32, tag="r")
                nc.vector.reciprocal_approx_fast(r_t[:], d_t[:])
                F_t = small.tile([8, BLK, 128], f16, tag="F")
                nc.vector.tensor_mul(F_t[:], s["a"][:], r_t[:])
                s["F"] = F_t

            def pe_ex(blk):
                s = st[blk]
                fe_ps = pf.tile([128, BLK, 128], f32)
                nc.tensor.matmul(fe_ps[:], ex[:], s["F"][:],
                                 start=True, stop=True)
                s["fe"] = fe_ps

            def dve_out(blk):
                s = st.pop(blk)
                r0 = blk * BLK
                v_t = work.tile([128, BLK, 128], f16, tag="v")
                nc.vector.tensor_mul(v_t[:], s["p_sb"][:], s["fe"][:])
                nc.sync.dma_start(out=out_v[:, r0:r0 + BLK, :], in_=v_t[:])

            for b in range(N_BLKS + 3):
                if 1 <= b <= N_BLKS:
                    act_sq(b - 1)
                if 2 <= b <= N_BLKS + 1:
                    act_sqrt(b - 2)
                if b < N_BLKS:
                    conv(b)
                if b in (0, 4, 8):
                    load_tile(b // 4 + 1)
                if 1 <= b <= N_BLKS:
                    pe_bd(b - 1)
                if 2 <= b <= N_BLKS + 1:
                    dve_chain(b - 2)
                    pe_ex(b - 2)
                if 3 <= b:
                    dve_out(b - 3)

    nc.compile()
    return nc


def _prep_in_maps(u, W):
    x = u.reshape(N, CIN, H, W_SP)
    wl = _weight_tiles(W).reshape(128, 13 * 128)
    bd = _block_diag()
    ex = np.ascontiguousarray(bd.T)
    in_maps = []
    for core in range(N_CORES):
        n, half = core // 2, core % 2
        XA, XC = _inputs_core(x[n], half)
        in_maps.append({
            "xa": XA.reshape(128, XROWS * XCOLS),
            "xc": XC.reshape(128, XROWS * XCOLS),
            "wl": wl,
            "bd": bd,
            "ex": ex,
            "cm": _c_map(half),
        })
    return in_maps


def run(u, W, trace=False):
    """Returns (out [N,T1,Z1,H,W] f32, BassKernelResults)."""
    from concourse.bass_utils import run_bass_kernel_spmd

    if "nc" not in _CACHE:
        _CACHE["nc"] = build_nc()
    nc = _CACHE["nc"]
    in_maps = _prep_in_maps(np.asarray(u, np.float32), np.asarray(W, np.float32))
    res = run_bass_kernel_spmd(nc, in_maps, list(range(N_CORES)), trace=trace)
    out = np.empty((N, T1, Z1, H, W_SP), np.float32)
    for core in range(N_CORES):
        n, half = core // 2, core % 2
        o = res.results[core]["out"].astype(np.float32).reshape(T1, Z1, ROWS, 128)
        out[n, :, :, half * 64:(half + 1) * 64, :] = o
    return out, res


def kernel(u, W):
    out, _ = run(u, W, trace=False)
    return out


# revision 18
# speedup vs baseline: 2.6277x; 1.0529x over previous
"""Trainium2 Bass kernel for nn_CapsuleLayer_9852654977072.

The reference module collapses mathematically: the routing loop's coupling
logits `b` stay zero (faithfully-reproduced bug in the original torch code),
so routing coefficients are a fixed spatial map s(h,w) = 1/(8*cnt(h,w)) where
cnt is the 5x5 box-count inside the image. The whole module is therefore:

    praw = conv2d(u as [N,64,H,W], Wd as [128,64,5,5], pad=2)
    v    = praw * sqrt(u2) / (c + u2)        # u2 = sum_z1 praw^2 (groups of 16)
    out[n,t1,z1,h,w] = v                     # c = 1/s^2 spatial map

(The eps inside the reference's sqrt(n2+1e-9) is negligible at these
magnitudes; n2/sqrt(n2+eps) == sqrt(n2) to ~1e-7 relative.)

Device strategy (8 cores, SPMD): shard (batch n in 0..3) x (row-half in 0..1).
Each core computes all 128 output channels for 64 rows of one image.

Conv: inputs shipped as XA/XC fp16 whose partition halves hold u shifted by
(+0row,+1row) and (+2row+0col,+2row+1col), columns padded by 2, stored as 4
overlapping 20-row SBUF tiles so block b depends only on tile b//4.  Per
4-row block, 13 PSUM-accumulated fp16 matmuls (N=512, full PE rate, all
K=128 so FWL hides LDWEIGHTS; tap 13 zero-padded) cover all 25 taps.

Squash: square (ACT, ->fp16) -> "fat block-diag" matmul bd128[k,m] =
(k//16==m//16) which yields u2 already broadcast over all 128 channel
partitions -> sqrt [ACT] / +c, reciprocal, mul [DVE] -> v = praw*F [DVE,
all-fp16] -> fp16 DMA out (host casts back to fp32).  No expand matmul.

Software pipeline per step b (engine program order):
    ACT: sq(b-1), cp(b-1), sqrt(b-3)
    PE : conv(b) x13, bd(b-2)
    DVE: add(b-3), recip(b-3), F(b-3), v(b-3)
Late input tiles are DMA'd behind an ACT-ordered dummy read so they don't
round-robin-steal HBM bandwidth from tile 0 during startup; dummy PE matmuls
warm the HAM clock gate during the initial DMA fill.
"""

import numpy as np

T0, Z0, T1, Z1, KK, PAD = 4, 16, 8, 16, 5, 2
N, H, W_SP = 4, 128, 128
CIN, COUT = T0 * Z0, T1 * Z1  # 64, 128
N_CORES = 8
ROWS = 64          # output rows per core
XROWS = 68         # input rows incl. halo
XCOLS = 132        # 128 + 2*PAD
BLK = 4            # output rows per block
N_BLKS = ROWS // BLK
TROWS = 20         # rows per input SBUF tile (16 + 4 overlap)

# conv matmul j -> (source, row_off, col_off); weights match in _weight_tiles
_MM_SLICES = (
    [('XA', dy + 2, dx + 2) for dy in (-2, 0) for dx in (-2, -1, 0, 1, 2)]
    + [('XC', 2, 0), ('XC', 2, 2), ('XC', 2, 4)]
)

_CACHE = {}


def _weight_tiles(W):
    Wd = W.transpose(1, 0, 2, 3, 4).reshape(COUT, CIN, KK, KK)
    wl = np.zeros((128, 13, 128), np.float32)  # [k, j, m]
    j = 0
    for dy in (-2, 0):
        for dx in (-2, -1, 0, 1, 2):
            wl[0:64, j, :] = Wd[:, :, dy + 2, dx + 2].T
            wl[64:128, j, :] = Wd[:, :, dy + 3, dx + 2].T
            j += 1
    for dx0 in (-2, 0):
        wl[0:64, j, :] = Wd[:, :, 4, dx0 + 2].T
        wl[64:128, j, :] = Wd[:, :, 4, dx0 + 3].T
        j += 1
    wl[0:64, j, :] = Wd[:, :, 4, 4].T  # single tap (2,2); hi partitions stay 0
    return wl.astype(np.float16)


def _inputs_core(x, half):
    """x: [64, H, W] one image channel-major. Returns XA, XC fp16 [128,68,132]."""
    base = half * 64 - 2
    XA = np.zeros((128, XROWS, XCOLS), np.float16)
    XC = np.zeros((128, XROWS, XCOLS), np.float16)

    def fill(dst, roff, c0, c1):
        lo, hi = max(0, -(base + roff)), min(XROWS, H - base - roff)
        dst[:, lo:hi, c0:c1] = x[:, base + roff + lo:base + roff + hi, :]

    fill(XA[0:64], 0, 2, 130)
    fill(XA[64:128], 1, 2, 130)
    fill(XC[0:64], 2, 2, 130)
    fill(XC[64:128], 2, 1, 129)
    return XA, XC


def _c_map(half):
    """c(h,w) = 1/s^2 = (8*cnt_r*cnt_c)^2 for this core's row half."""
    idx = np.arange(H)
    cnt = (np.minimum(idx + 2, H - 1) - np.maximum(idx - 2, 0) + 1).astype(np.float64)
    c = (8.0 * cnt[:, None] * cnt[None, :]) ** 2  # [H, W]
    c = c[half * 64:(half + 1) * 64, :]
    return np.ascontiguousarray(c.astype(np.float32).reshape(1, ROWS * 128))


def _block_diag128():
    k = np.arange(128)
    return ((k[:, None] // 16) == (k[None, :] // 16)).astype(np.float16)


def build_nc():
    import concourse.bass as bass
    import concourse.bacc as bacc
    import concourse.mybir as mybir
    import concourse.tile as tile

    f32 = mybir.dt.float32
    f16 = mybir.dt.float16
    AF = mybir.ActivationFunctionType

    nc = bacc.Bacc(None, target_bir_lowering=False)
    xa_d = nc.dram_tensor("xa", [128, XROWS * XCOLS], f16, kind="ExternalInput")
    xc_d = nc.dram_tensor("xc", [128, XROWS * XCOLS], f16, kind="ExternalInput")
    wl_d = nc.dram_tensor("wl", [128, 13 * 128], f16, kind="ExternalInput")
    bd_d = nc.dram_tensor("bd", [128, 128], f16, kind="ExternalInput")
    cm_d = nc.dram_tensor("cm", [1, ROWS * 128], f32, kind="ExternalInput")
    out_d = nc.dram_tensor("out", [128, ROWS * 128], f16, kind="ExternalOutput")

    with tile.TileContext(nc) as tc:
        with (
            tc.tile_pool(name="consts", bufs=1) as consts,
            tc.tile_pool(name="work", bufs=4) as work,
            tc.tile_pool(name="small", bufs=4) as small,
            tc.tile_pool(name="pp", bufs=4, space="PSUM") as pp,
            tc.tile_pool(name="py", bufs=4, space="PSUM") as py,
        ):
            xa_src = xa_d.ap().rearrange("p (r c) -> p r c", c=XCOLS)
            xc_src = xc_d.ap().rearrange("p (r c) -> p r c", c=XCOLS)

            # startup-critical loads first: wl + input tile 0
            wl = consts.tile([128, 13, 128], f16)
            nc.sync.dma_start(
                out=wl, in_=wl_d.ap().rearrange("p (j m) -> p j m", m=128))
            xa_t = []
            xc_t = []
            for k in range(4):
                xa_t.append(consts.tile([128, TROWS, XCOLS], f16,
                                        name=f"xa{k}", tag=f"xa{k}"))
                xc_t.append(consts.tile([128, TROWS, XCOLS], f16,
                                        name=f"xc{k}", tag=f"xc{k}"))
            nc.sync.dma_start(out=xa_t[0], in_=xa_src[:, 0:TROWS, :])
            nc.sync.dma_start(out=xc_t[0], in_=xc_src[:, 0:TROWS, :])
            bd = consts.tile([128, 128], f16)
            nc.sync.dma_start(out=bd, in_=bd_d.ap())
            cm_sb = consts.tile([128, ROWS, 128], f32)
            cm_ap = cm_d.ap()
            nc.sync.dma_start(
                out=cm_sb,
                in_=bass.AP(tensor=cm_ap.tensor, offset=cm_ap.offset,
                            ap=[[0, 128], [128, ROWS], [1, 128]]))

            dum = consts.tile([1, 8], f16)

            def load_tile(k):
                # ACT-ordered dummy reads give the DMAs a WAR dependency, so
                # they fire mid-loop instead of stealing round-robin HBM
                # bandwidth from tile 0 / wl during startup.
                nc.scalar.activation(dum[0:1, 0:1], xa_t[k][0:1, 0, 0:1],
                                     AF.Copy, bias=0.0)
                nc.scalar.activation(dum[0:1, 1:2], xc_t[k][0:1, 0, 0:1],
                                     AF.Copy, bias=0.0)
                r0 = 16 * k
                nc.sync.dma_start(out=xa_t[k], in_=xa_src[:, r0:r0 + TROWS, :])
                nc.sync.dma_start(out=xc_t[k], in_=xc_src[:, r0:r0 + TROWS, :])

            out_v = out_d.ap().rearrange("p (r c) -> p r c", c=128)

            # HAM warm-up: dummy matmuls during the input-DMA fill so the PE
            # clock gate opens before the first real conv block.
            wtmp = consts.tile([128, 256], f16)
            nc.vector.memset(wtmp[:], 0.125)
            pwarm = pp.tile([128, BLK, 128], f32, tag="p_ps")
            for _ in range(14):
                nc.tensor.matmul(pwarm[:, 0:2, :], wtmp[:, 0:128], wtmp[:],
                                 start=True, stop=True)

            st = {}

            def conv(blk):
                lr0 = (blk % 4) * BLK
                xak, xck = xa_t[blk // 4], xc_t[blk // 4]
                p_ps = pp.tile([128, BLK, 128], f32, tag="p_ps")
                for j, (src, roff, coff) in enumerate(_MM_SLICES):
                    xsrc = xak if src == 'XA' else xck
                    nc.tensor.matmul(
                        p_ps[:], wl[:, j, :],
                        xsrc[:, lr0 + roff:lr0 + roff + BLK, coff:coff + 128],
                        start=(j == 0), stop=(j == 12))
                st[blk] = {"p": p_ps}

            def act_sq(blk):
                s = st[blk]
                psq = work.tile([128, BLK, 128], f16, tag="psq")
                nc.scalar.activation(psq[:], s["p"][:], AF.Square)
                p_sb = work.tile([128, BLK, 128], f16, tag="p_sb")
                nc.scalar.activation(p_sb[:], s["p"][:], AF.Copy, bias=0.0)
                s["psq"], s["p_sb"] = psq, p_sb

            def pe_bd(blk):
                s = st[blk]
                y_ps = py.tile([128, BLK, 128], f32)
                nc.tensor.matmul(y_ps[:], bd[:], s["psq"][:],
                                 start=True, stop=True)
                s["y"] = y_ps

            def act_sqrt(blk):
                s = st[blk]
                a_t = small.tile([128, BLK, 128], f32, tag="a")
                nc.scalar.activation(a_t[:], s["y"][:], AF.Sqrt)
                s["a"] = a_t

            def dve_rest(blk):
                s = st.pop(blk)
                r0 = blk * BLK
                # F = sqrt(u2) / (c + u2); v = praw * F
                d_t = small.tile([128, BLK, 128], f32, tag="d")
                nc.vector.tensor_add(d_t[:], s["y"][:], cm_sb[:, r0:r0 + BLK, :])
                r_t = small.tile([128, BLK, 128], f32, tag="r")
                nc.vector.reciprocal_approx_fast(r_t[:], d_t[:])
                F_t = small.tile([128, BLK, 128], f16, tag="F")
                nc.vector.tensor_mul(F_t[:], s["a"][:], r_t[:])
                v_t = work.tile([128, BLK, 128], f16, tag="v")
                nc.vector.tensor_mul(v_t[:], s["p_sb"][:], F_t[:])
                nc.sync.dma_start(out=out_v[:, r0:r0 + BLK, :], in_=v_t[:])

            for b in range(N_BLKS + 4):
                if 1 <= b <= N_BLKS:
                    act_sq(b - 1)
                if b in (1, 4, 8):
                    load_tile(b // 4 + 1)
                if 3 <= b <= N_BLKS + 2:
                    act_sqrt(b - 3)
                if b < N_BLKS:
                    conv(b)
                if 2 <= b <= N_BLKS + 1:
                    pe_bd(b - 2)
                if 3 <= b <= N_BLKS + 2:
                    dve_rest(b - 3)

    nc.compile()
    return nc


def _prep_in_maps(u, W):
    x = u.reshape(N, CIN, H, W_SP)
    wl = _weight_tiles(W).reshape(128, 13 * 128)
    bd = _block_diag128()
    in_maps = []
    for core in range(N_CORES):
        n, half = core // 2, core % 2
        XA, XC = _inputs_core(x[n], half)
        in_maps.append({
            "xa": XA.reshape(128, XROWS * XCOLS),
            "xc": XC.reshape(128, XROWS * XCOLS),
            "wl": wl,
            "bd": bd,
            "cm": _c_map(half),
        })
    return in_maps


def run(u, W, trace=False):
    """Returns (out [N,T1,Z1,H,W] f32, BassKernelResults)."""
    from concourse.bass_utils import run_bass_kernel_spmd

    if "nc" not in _CACHE:
        _CACHE["nc"] = build_nc()
    nc = _CACHE["nc"]
    in_maps = _prep_in_maps(np.asarray(u, np.float32), np.asarray(W, np.float32))
    res = run_bass_kernel_spmd(nc, in_maps, list(range(N_CORES)), trace=trace)
    out = np.empty((N, T1, Z1, H, W_SP), np.float32)
    for core in range(N_CORES):
        n, half = core // 2, core % 2
        o = res.results[core]["out"].astype(np.float32).reshape(T1, Z1, ROWS, 128)
        out[n, :, :, half * 64:(half + 1) * 64, :] = o
    return out, res


def kernel(u, W):
    out, _ = run(u, W, trace=False)
    return out
